# revision 1
# baseline (speedup 1.0000x reference)
"""Trainium2 Bass kernel: teacher-forced GRU decoder (B=512, T=32, H=2048, E=4096).

Sharding: pure data-parallel over batch across 8 NeuronCores (64 seqs/core).
Per-core dataflow (everything feature-on-partitions, "transposed" layouts):
  Phase A: GX^T[j, t*BL+b] = W_ih @ X^T (+ b_ih, + b_hh for r/z rows), all
           timesteps batched -> DRAM scratch, fp16.
  Phase B: 32-step scan. Weight-stationary matmuls: lhsT = W_hh^T tiles
           (15/16 resident in SBUF, 1 re-streamed), rhs = h^T (fp16).
           Gate math on DVE/ACT; fp32 master h lives in 2 spare PSUM banks.
  Phase C: logits = Hall^T.T @ W_out^T + b_out, log_softmax along E (free dim),
           batched over all (t, b) rows.
"""

import os
import sys

for _p in ("/opt/trn_rl_repo", "/root/.axon_site/_ro/trn_rl_repo"):
    if os.path.isdir(_p) and _p not in sys.path:
        sys.path.append(_p)

import numpy as np

import concourse.bass as bass
import concourse.mybir as mybir
import concourse.tile as tile
from concourse import bacc

F16 = mybir.dt.float16
F32 = mybir.dt.float32
AF = mybir.ActivationFunctionType
OP = mybir.AluOpType

NCORES = 8


def build(BL=64, T=32, H=2048, E=4096, whh_resident=16):
    """Build the single-core Bass program (SPMD across cores)."""
    G3 = 3 * H
    TB = T * BL
    KH = H // 128          # h contraction tiles (16)
    KH2 = KH // 2          # half split (8)
    KE = E // 128          # e contraction tiles (32)
    M3 = G3 // 128         # gate-row tiles (48)
    MC = TB // 128         # phase-C row tiles (16)
    assert 128 % BL == 0
    SPM = 128 // BL        # steps per phase-C m tile (2)
    PA_N = 512 if TB % 512 == 0 else TB   # phase-A moving tile
    NT_A = TB // PA_N      # phase-A n tiles
    SPN = PA_N // BL       # steps per phase-A n tile
    CHUNK = min(E, 2048)   # phase-C psum chunk (4 banks)
    NCHUNK = E // CHUNK
    NT_CC = CHUNK // 512   # 512-slices per chunk
    R = min(whh_resident, KH)
    NSTREAM = KH - R
    assert KH2 * BL == 512

    nc = bacc.Bacc(target_bir_lowering=False, trn_type="TRN2")

    xT = nc.declare_dram_parameter("xT", [E, TB], F16, isOutput=False)
    wihT = nc.declare_dram_parameter("wihT", [E, G3], F16, isOutput=False)
    whhT = nc.declare_dram_parameter("whhT", [H, G3], F16, isOutput=False)
    woutT = nc.declare_dram_parameter("woutT", [H, E], F16, isOutput=False)
    h0T = nc.declare_dram_parameter("h0T", [H, BL], F16, isOutput=False)
    biasA = nc.declare_dram_parameter("biasA", [128, M3], F32, isOutput=False)
    bhhN = nc.declare_dram_parameter("bhhN", [128, KH], F32, isOutput=False)
    boutT = nc.declare_dram_parameter("boutT", [1, E], F16, isOutput=False)
    out_d = nc.declare_dram_parameter("out", [TB, E], F32, isOutput=True)

    gx_d = nc.dram_tensor("gx_scratch", [T, M3, 128, BL], F16)
    hall_d = nc.dram_tensor("hall_scratch", [T, KH, 128, BL], F16)

    wihT_p = wihT[:].rearrange("(k p) j -> p k j", p=128)     # (128, KE, G3)
    whhT_p = whhT[:].rearrange("(k p) j -> p k j", p=128)     # (128, KH, G3)
    woutT_p = woutT[:].rearrange("(k p) j -> p k j", p=128)   # (128, KH, E)
    xT_p = xT[:].rearrange("(k p) n -> p k n", p=128)         # (128, KE, TB)
    h0T_p = h0T[:].rearrange("(k p) b -> p k b", p=128)       # (128, KH, BL)

    with tile.TileContext(nc) as tc:
        # ================= Phase A: GX = W_ih @ X^T =================
        with tc.tile_pool(name="phaseA", bufs=1) as pa_single, \
             tc.tile_pool(name="pa_w", bufs=2) as pa_w, \
             tc.tile_pool(name="pa_g", bufs=4) as pa_g, \
             tc.tile_pool(name="pa_ps", bufs=8, space="PSUM") as pa_ps:
            ba_sb = pa_single.tile([128, M3], F32)
            nc.sync.dma_start(out=ba_sb, in_=biasA[:])
            x_sb = [pa_single.tile([128, TB], F16, tag=f"x{k}", name=f"x{k}")
                    for k in range(KE)]
            for k in range(KE):
                nc.sync.dma_start(out=x_sb[k], in_=xT_p[:, k])
            for m in range(M3):
                wsl = pa_w.tile([128, KE, 128], F16, tag="wsl")
                nc.sync.dma_start(out=wsl, in_=wihT_p[:, :, m * 128:(m + 1) * 128])
                for n in range(NT_A):
                    ps = pa_ps.tile([128, PA_N], F32, tag="pa_psum")
                    for k in range(KE):
                        nc.tensor.matmul(
                            ps,
                            wsl[:, k, :],
                            x_sb[k][:, n * PA_N:(n + 1) * PA_N],
                            start=(k == 0),
                            stop=(k == KE - 1),
                        )
                    g = pa_g.tile([128, PA_N], F16, tag="gstage")
                    nc.vector.tensor_scalar_add(g, ps, ba_sb[:, m:m + 1])
                    nc.sync.dma_start(
                        out=gx_d[n * SPN:(n + 1) * SPN, m].rearrange("t p b -> p t b"),
                        in_=g.rearrange("p (t b) -> p t b", b=BL),
                    )

        # ================= Phase B: GRU scan =================
        with tc.tile_pool(name="sc_small", bufs=1) as sc_small, \
             tc.tile_pool(name="whh_res", bufs=1) as whh_pool, \
             tc.tile_pool(name="whh_stream", bufs=1) as whh_sp, \
             tc.tile_pool(name="h16p", bufs=2) as h16_pool, \
             tc.tile_pool(name="gxs", bufs=2) as gxs_pool, \
             tc.tile_pool(name="gate", bufs=1) as gate_pool, \
             tc.tile_pool(name="hops", bufs=2) as hops_pool, \
             tc.tile_pool(name="h32ps", bufs=1, space="PSUM") as h32_ps, \
             tc.tile_pool(name="sc_ps", bufs=6, space="PSUM") as sc_ps:

            bn_sb = sc_small.tile([128, KH], F32)
            nc.sync.dma_start(out=bn_sb, in_=bhhN[:])

            whh_sb = [whh_pool.tile([128, G3], F16, tag=f"whh{k}",
                                    name=f"whh{k}")
                      for k in range(R)]
            for k in range(R):
                nc.sync.dma_start(out=whh_sb[k], in_=whhT_p[:, k])
            wstream = [whh_sp.tile([128, G3], F16, tag=f"wst{i}",
                                   name=f"wst{i}")
                       for i in range(NSTREAM)]

            # fp32 master h lives in 2 PSUM banks (one per half); h0 arrives
            # as f16, is DMA'd to SBUF, then upcast-copied into PSUM.
            h16_prev = h16_pool.tile([128, KH, BL], F16, tag="h16")
            nc.sync.dma_start(out=h16_prev, in_=h0T_p)
            h32 = []
            for hf in range(2):
                hb = h32_ps.tile([128, KH2 * BL], F32, tag=f"h32_{hf}",
                                 name=f"h32_{hf}")
                nc.vector.tensor_copy(
                    out=hb.rearrange("p (k b) -> p k b", b=BL),
                    in_=h16_prev[:, hf * KH2:(hf + 1) * KH2])
                h32.append(hb)

            gx_t = gx_d[:].rearrange("t (g k) p b -> t g k p b", g=3)

            def whh_tile(k):
                if k < R:
                    return whh_sb[k]
                return wstream[k - R]

            for t in range(T):
                # stream the non-resident W_hh k-tiles (same data each step;
                # re-DMA so the SBUF slot is reusable -- WAR handled by Tile)
                for i in range(NSTREAM):
                    nc.sync.dma_start(
                        out=wstream[i], in_=whhT_p[:, R + i, :])

                h16_cur = h16_pool.tile([128, KH, BL], F16, tag="h16")
                # prefetch gx for both halves of this step
                gxs = []
                for hf in range(2):
                    gt = gxs_pool.tile([128, 3, KH2, BL], F16, tag="gxs")
                    for g in range(3):
                        nc.sync.dma_start(
                            out=gt[:, g],
                            in_=gx_t[t, g, hf * KH2:(hf + 1) * KH2].rearrange(
                                "k p b -> p k b"),
                        )
                    gxs.append(gt)

                for hf in range(2):
                    ps_gate = [sc_ps.tile([128, KH2 * BL], F32, tag="sc_psum",
                                          name=f"ps{g}")
                               for g in range(3)]
                    # Bank-level accumulation groups (one start/stop per psum
                    # bank, first-touch overwrite via has_written): two
                    # kappa-passes so this step's first-half matmuls only
                    # need h[0:KH2] -- the previous step's gating tail for
                    # h[KH2:] overlaps with pass 1.
                    for kp in range(2):
                        for g in range(3):
                            ps = ps_gate[g]
                            for s in range(KH2):
                                m = g * KH + hf * KH2 + s
                                for k in range(kp * KH2, (kp + 1) * KH2):
                                    nc.tensor.matmul(
                                        ps[:, s * BL:(s + 1) * BL],
                                        whh_tile(k)[:, m * 128:(m + 1) * 128],
                                        h16_prev[:, k, :],
                                        start=(kp == 0 and s == 0 and k == 0),
                                        stop=(kp == 1 and s == KH2 - 1
                                              and k == KH - 1),
                                        skip_group_check=True,
                                    )
                    ks = slice(hf * KH2, (hf + 1) * KH2)
                    gx_h = gxs[hf]
                    psr = ps_gate[0].rearrange("p (s b) -> p s b", b=BL)
                    psz = ps_gate[1].rearrange("p (s b) -> p s b", b=BL)
                    psn = ps_gate[2]
                    h32h = h32[hf].rearrange("p (k b) -> p k b", b=BL)
                    # r / z gates: preact in-place onto gx, sigmoid on ACT
                    nc.vector.tensor_add(gx_h[:, 0], psr, gx_h[:, 0])
                    r_h = gate_pool.tile([128, KH2, BL], F16, tag="r_h")
                    nc.scalar.activation(out=r_h, in_=gx_h[:, 0],
                                         func=AF.Sigmoid)
                    nc.vector.tensor_add(gx_h[:, 1], psz, gx_h[:, 1])
                    z_h = gate_pool.tile([128, KH2, BL], F16, tag="z_h")
                    nc.scalar.activation(out=z_h, in_=gx_h[:, 1],
                                         func=AF.Sigmoid)
                    # n gate: tanh(gx_n + r * (gh_n + bhh_n))
                    for s in range(KH2):
                        kg = hf * KH2 + s
                        nc.vector.scalar_tensor_tensor(
                            out=psn[:, s * BL:(s + 1) * BL],
                            in0=psn[:, s * BL:(s + 1) * BL],
                            scalar=bn_sb[:, kg:kg + 1],
                            in1=r_h[:, s, :],
                            op0=OP.add,
                            op1=OP.mult,
                        )
                    nc.vector.tensor_add(
                        gx_h[:, 2],
                        psn.rearrange("p (s b) -> p s b", b=BL),
                        gx_h[:, 2])
                    n_h = gate_pool.tile([128, KH2, BL], F16, tag="n_h")
                    nc.scalar.activation(out=n_h, in_=gx_h[:, 2],
                                         func=AF.Tanh)
                    # h' = n + z * (h - n)
                    t4 = hops_pool.tile([128, KH2, BL], F16, tag="t4")
                    nc.vector.tensor_sub(t4, h32h, n_h)
                    nc.vector.tensor_mul(t4, z_h, t4)
                    nc.vector.tensor_add(h32h, n_h, t4)
                    nc.vector.tensor_copy(out=h16_cur[:, ks], in_=h32h)

                nc.sync.dma_start(
                    out=hall_d[t].rearrange("k p b -> p k b"), in_=h16_cur)
                h16_prev = h16_cur

        # ================= Phase C: logits + log_softmax =================
        with tc.tile_pool(name="c_small", bufs=1) as c_small, \
             tc.tile_pool(name="wout_res", bufs=1) as wo_pool, \
             tc.tile_pool(name="hall_in", bufs=2) as hall_pool, \
             tc.tile_pool(name="logits", bufs=2) as lg_pool, \
             tc.tile_pool(name="expbuf", bufs=2) as ex_pool, \
             tc.tile_pool(name="stats", bufs=8) as st_pool, \
             tc.tile_pool(name="c_ps", bufs=2, space="PSUM") as c_ps:
            bo_sb = c_small.tile([1, E], F16)
            nc.sync.dma_start(out=bo_sb, in_=boutT[:])
            ones_sb = c_small.tile([1, 128], F16)
            nc.vector.memset(ones_sb, 1.0)
            wo_sb = [wo_pool.tile([128, E], F16, tag=f"wo{k}", name=f"wo{k}")
                     for k in range(KH)]
            for k in range(KH):
                nc.sync.dma_start(out=wo_sb[k], in_=woutT_p[:, k])
            for m in range(MC):
                hs = hall_pool.tile([128, KH, SPM, BL], F16, tag="hs")
                for tp in range(SPM):
                    nc.sync.dma_start(
                        out=hs[:, :, tp, :],
                        in_=hall_d[m * SPM + tp].rearrange("k p b -> p k b"),
                    )
                lg = lg_pool.tile([128, E], F32, tag="lg")
                for c in range(NCHUNK):
                    ps = c_ps.tile([128, CHUNK], F32, tag="c_psum")
                    for nt in range(NT_CC):
                        nsl = slice(nt * 512, (nt + 1) * 512)
                        nglob = c * CHUNK + nt * 512
                        for k in range(KH):
                            nc.tensor.matmul(
                                ps[:, nsl],
                                hs[:, k].rearrange("p t b -> p (t b)"),
                                wo_sb[k][:, nglob:nglob + 512],
                                start=(k == 0),
                                stop=False,
                            )
                        nc.tensor.matmul(
                            ps[:, nsl],
                            ones_sb,
                            bo_sb[:, nglob:nglob + 512],
                            start=False,
                            stop=True,
                        )
                    nc.vector.tensor_copy(
                        out=lg[:, c * CHUNK:(c + 1) * CHUNK], in_=ps)
                negmax = st_pool.tile([128, 1], F32, tag="negmax")
                nc.vector.tensor_reduce(
                    out=negmax, in_=lg, axis=mybir.AxisListType.X,
                    op=OP.max, negate=True)
                eb = ex_pool.tile([128, E], F16, tag="eb")
                sumexp = st_pool.tile([128, 1], F32, tag="sumexp")
                nc.scalar.activation(
                    out=eb, in_=lg, func=AF.Exp,
                    bias=negmax, scale=1.0, accum_out=sumexp)
                lse = st_pool.tile([128, 1], F32, tag="lse")
                nc.scalar.activation(out=lse, in_=sumexp, func=AF.Ln)
                negoff = st_pool.tile([128, 1], F32, tag="negoff")
                nc.vector.tensor_sub(negoff, negmax, lse)
                nc.vector.tensor_scalar_add(lg, lg, negoff)
                nc.sync.dma_start(
                    out=out_d[m * 128:(m + 1) * 128, :], in_=lg)

    nc.finalize()
    return nc


def _host_prep(context_batch, target_encs, sos, W_ih, W_hh, b_ih, b_hh,
               W_out, b_out, BL, T, H, E):
    """Build per-core input maps (numpy layout transforms only)."""
    G3 = 3 * H
    M3 = G3 // 128
    KH = H // 128
    B = context_batch.shape[0]
    ncores = B // BL

    wihT = np.ascontiguousarray(W_ih.T).astype(np.float16)
    whhT = np.ascontiguousarray(W_hh.T).astype(np.float16)
    woutT = np.ascontiguousarray(W_out.T).astype(np.float16)
    biasA = b_ih.astype(np.float32).copy()
    biasA[:2 * H] += b_hh[:2 * H].astype(np.float32)
    biasA = np.ascontiguousarray(biasA.reshape(M3, 128).T)
    bhhN = np.ascontiguousarray(
        b_hh[2 * H:].astype(np.float32).reshape(KH, 128).T)
    boutT = b_out.astype(np.float16).reshape(1, E)

    in_maps = []
    for c in range(ncores):
        sl = slice(c * BL, (c + 1) * BL)
        # teacher-forced inputs: SOS, then targets 0..T-2
        xc = np.empty((BL, T, E), np.float32)
        xc[:, 0, :] = sos
        xc[:, 1:, :] = target_encs[sl, :T - 1, :]
        # (E, T*BL) with column index t*BL + b
        xT = np.ascontiguousarray(
            xc.transpose(2, 1, 0).reshape(E, T * BL)).astype(np.float16)
        h0T = np.ascontiguousarray(
            context_batch[sl].T).astype(np.float16)
        in_maps.append({
            "xT": xT, "wihT": wihT, "whhT": whhT, "woutT": woutT,
            "h0T": h0T, "biasA": biasA, "bhhN": bhhN, "boutT": boutT,
        })
    return in_maps


_CACHE = {}


def kernel(context_batch, target_encs, sos, W_ih, W_hh, b_ih, b_hh,
           W_out, b_out, trace=False):
    B, T, E = target_encs.shape
    H = context_batch.shape[1]
    BL = B // NCORES

    if "nc" not in _CACHE:
        _CACHE["nc"] = build(BL=BL, T=T, H=H, E=E)
    nc = _CACHE["nc"]

    in_maps = _host_prep(context_batch, target_encs, sos, W_ih, W_hh,
                         b_ih, b_hh, W_out, b_out, BL, T, H, E)

    from concourse.bass_utils import run_bass_kernel_spmd
    res = run_bass_kernel_spmd(nc, in_maps, list(range(NCORES)), trace=trace)

    outs = []
    for c in range(NCORES):
        o = res.results[c]["out"]            # (T*BL, E), row = t*BL + b
        outs.append(o.reshape(T, BL, E).transpose(1, 0, 2))
    full = np.concatenate(outs, axis=0).astype(np.float32)
    if trace:
        _CACHE["last_exec_time_ns"] = res.exec_time_ns
    return full



# revision 3
# speedup vs baseline: 3.4764x; 3.4764x over previous
"""Trainium2 Bass kernel: teacher-forced GRU decoder (B=512, T=32, H=2048, E=4096).

Sharding: pure data-parallel over batch across 8 NeuronCores (64 seqs/core).
All three GEMM phases run in fp8(e4m3) DoubleRow mode (2x PE throughput);
accumulation is fp32 in PSUM, gate math fp16/fp32 on DVE/ACT, log-softmax fp32.

Power-of-2 scaling keeps fp8 operands in the format's sweet spot:
  weights x 2^9 (sigma 0.02 -> ~10), x x 2^7 ([0,1) -> [0,128)),
  h x 2^5 (|h| <~ 4.5 -> <~ 150; e4m3 max 240).
PSUM therefore holds preactivations x 2^16 (phase A) / x 2^14 (B, C); the
descale folds into the existing psum-evacuation ops for free.

Per-core dataflow (feature-on-partitions "transposed" layouts):
  Phase A: GX^T = W_ih @ X^T (+biases) for all timesteps -> DRAM, fp16.
  Phase B: 32-step scan; W_hh fp8 fully SBUF-resident; fp32 master h in PSUM;
           h8 (fp8) regenerated each step for the next matmul.
  Phase C: logits = Hall^T.T @ W_out^T + b_out, log_softmax along E.

DoubleRow contraction layout: k_global = k2*256 + i*128 + p for lhsT/rhs
tiles indexed [p, k2, i, ...]; equivalently kk = 2*k2 + i for the legacy
[p, kk, ...] 128-row tiling (pure relabeling, same memory order).
"""

import os
import sys

for _p in ("/opt/trn_rl_repo", "/root/.axon_site/_ro/trn_rl_repo"):
    if os.path.isdir(_p) and _p not in sys.path:
        sys.path.append(_p)

import numpy as np
import ml_dtypes

import concourse.bass as bass
import concourse.mybir as mybir
import concourse.tile as tile
from concourse import bacc

F8 = mybir.dt.float8e4
F16 = mybir.dt.float16
F32 = mybir.dt.float32
AF = mybir.ActivationFunctionType
OP = mybir.AluOpType
DR = mybir.MatmulPerfMode.DoubleRow

NCORES = 8
NP8 = ml_dtypes.float8_e4m3

SW = 2.0 ** 9    # weight scale
SX = 2.0 ** 7    # x scale
SH = 2.0 ** 5    # h scale
ISA = 2.0 ** -16  # gx psum descale (SW*SX)
ISB = 2.0 ** -14  # gh / logits psum descale (SW*SH)


def build(BL=64, T=32, H=2048, E=4096):
    """Build the single-core Bass program (SPMD across cores)."""
    G3 = 3 * H
    TB = T * BL
    KH = H // 128          # 128-row h contraction tiles (16)
    KH2 = KH // 2          # fp8 k2 (256-row) h tiles (8)
    KH2H = KH2 // 2        # k2 tiles per half (4)
    KE2 = E // 256         # fp8 k2 e tiles (16)
    M3 = G3 // 128         # gate-row tiles (48)
    MC = TB // 128         # phase-C row tiles (16)
    SPM = 128 // BL        # steps per phase-C m tile (2)
    PA_N = 512 if TB % 512 == 0 else TB
    NT_A = TB // PA_N      # phase-A n tiles (4)
    SPN = PA_N // BL       # steps per phase-A n tile (8)
    CHUNK = min(E, 2048)   # phase-C psum chunk (4 banks)
    NCHUNK = E // CHUNK
    NT_CC = CHUNK // 512

    nc = bacc.Bacc(target_bir_lowering=False, trn_type="TRN2")

    x8 = nc.declare_dram_parameter("x8", [128, KE2 * 2 * TB], F8,
                                   isOutput=False)
    wih8 = nc.declare_dram_parameter("wih8", [M3, 128, KE2 * 2 * 128], F8,
                                     isOutput=False)
    whh8 = nc.declare_dram_parameter("whh8", [128, KH2 * 2 * G3], F8,
                                     isOutput=False)
    wout8 = nc.declare_dram_parameter("wout8", [128, KH2 * 2 * E], F8,
                                      isOutput=False)
    h0T = nc.declare_dram_parameter("h0T", [128, KH * BL], F16,
                                    isOutput=False)
    biasA = nc.declare_dram_parameter("biasA", [128, M3], F32, isOutput=False)
    bn14 = nc.declare_dram_parameter("bn14", [128, KH], F32, isOutput=False)
    bo14 = nc.declare_dram_parameter("bo14", [1, E], F16, isOutput=False)
    out_d = nc.declare_dram_parameter("out", [TB, E], F32, isOutput=True)

    gx_d = nc.dram_tensor("gx_scratch", [T, M3, 128, BL], F16)
    hall_d = nc.dram_tensor("hall_scratch", [T, 128, KH * BL], F8)

    x8_p = x8[:].rearrange("p (k i n) -> p k i n", k=KE2, i=2)
    whh8_p = whh8[:].rearrange("p (k i j) -> p k i j", k=KH2, i=2)
    wout8_p = wout8[:].rearrange("p (k i e) -> p k i e", k=KH2, i=2)
    h0T_p = h0T[:].rearrange("p (k b) -> p k b", k=KH)

    with tile.TileContext(nc) as tc:
        # ================= Phase A: GX = W_ih @ X^T =================
        with tc.tile_pool(name="pa_x", bufs=1) as pa_x, \
             tc.tile_pool(name="pa_w", bufs=3) as pa_w, \
             tc.tile_pool(name="pa_g", bufs=4) as pa_g, \
             tc.tile_pool(name="pa_ps", bufs=8, space="PSUM") as pa_ps:
            ba_sb = pa_x.tile([128, M3], F32)
            nc.sync.dma_start(out=ba_sb, in_=biasA[:])
            x_sb = pa_x.tile([128, KE2, 2, TB], F8)
            nc.sync.dma_start(out=x_sb, in_=x8_p)
            for m in range(M3):
                wsl = pa_w.tile([128, KE2, 2, 128], F8, tag="wsl")
                nc.sync.dma_start(
                    out=wsl,
                    in_=wih8[m].rearrange("p (k i c) -> p k i c", k=KE2, i=2))
                for n in range(NT_A):
                    ps = pa_ps.tile([128, PA_N], F32, tag="pa_psum")
                    for k2 in range(KE2):
                        nc.tensor.matmul(
                            ps,
                            wsl[:, k2],
                            x_sb[:, k2, :, n * PA_N:(n + 1) * PA_N],
                            start=(k2 == 0),
                            stop=(k2 == KE2 - 1),
                            perf_mode=DR,
                        )
                    g = pa_g.tile([128, PA_N], F16, tag="gstage")
                    nc.vector.tensor_scalar(
                        out=g, in0=ps, scalar1=ISA,
                        scalar2=ba_sb[:, m:m + 1], op0=OP.mult, op1=OP.add)
                    nc.sync.dma_start(
                        out=gx_d[n * SPN:(n + 1) * SPN, m].rearrange(
                            "t p b -> p t b"),
                        in_=g.rearrange("p (t b) -> p t b", b=BL),
                    )

        # ================= Phase B: GRU scan =================
        with tc.tile_pool(name="sc_small", bufs=1) as sc_small, \
             tc.tile_pool(name="whh_res", bufs=1) as whh_pool, \
             tc.tile_pool(name="h8p", bufs=2) as h8_pool, \
             tc.tile_pool(name="gxs", bufs=2) as gxs_pool, \
             tc.tile_pool(name="gate", bufs=1) as gate_pool, \
             tc.tile_pool(name="hops", bufs=2) as hops_pool, \
             tc.tile_pool(name="h32ps", bufs=1, space="PSUM") as h32_ps, \
             tc.tile_pool(name="sc_ps", bufs=6, space="PSUM") as sc_ps:

            bn_sb = sc_small.tile([128, KH], F32)
            nc.sync.dma_start(out=bn_sb, in_=bn14[:])

            whh_sb = whh_pool.tile([128, KH2, 2, G3], F8)
            nc.sync.dma_start(out=whh_sb, in_=whh8_p)

            # fp32 master h lives in 2 PSUM banks (one per half); h0 arrives
            # as f16, is upcast into PSUM and downscaled-cast into fp8.
            h16_0 = sc_small.tile([128, KH, BL], F16)
            nc.sync.dma_start(out=h16_0, in_=h0T_p)
            h32 = []
            for hf in range(2):
                hb = h32_ps.tile([128, KH2 * BL], F32, tag=f"h32_{hf}",
                                 name=f"h32_{hf}")
                nc.vector.tensor_copy(
                    out=hb.rearrange("p (k b) -> p k b", b=BL),
                    in_=h16_0[:, hf * KH2:(hf + 1) * KH2])
                h32.append(hb)
            h8_prev = h8_pool.tile([128, KH, BL], F8, tag="h8")
            nc.vector.tensor_scalar_mul(h8_prev, h16_0, SH)

            gx_t = gx_d[:].rearrange("t (g k) p b -> t g k p b", g=3)

            for t in range(T):
                h8_cur = h8_pool.tile([128, KH, BL], F8, tag="h8")
                h8p_dr = h8_prev.rearrange("p (k i) b -> p k i b", i=2)
                # prefetch gx for both halves of this step
                gxs = []
                for hf in range(2):
                    gt = gxs_pool.tile([128, 3, KH2, BL], F16, tag="gxs")
                    for g in range(3):
                        nc.sync.dma_start(
                            out=gt[:, g],
                            in_=gx_t[t, g, hf * KH2:(hf + 1) * KH2].rearrange(
                                "k p b -> p k b"),
                        )
                    gxs.append(gt)

                for hf in range(2):
                    ps_gate = [sc_ps.tile([128, KH2 * BL], F32, tag="sc_psum",
                                          name=f"ps{g}")
                               for g in range(3)]
                    # Two kappa-passes: this step's first-half matmuls only
                    # need h[0:H/2], overlapping the previous step's tail.
                    for kp in range(2):
                        for g in range(3):
                            ps = ps_gate[g]
                            for s in range(KH2):
                                m = g * KH + hf * KH2 + s
                                for k2 in range(kp * KH2H, (kp + 1) * KH2H):
                                    nc.tensor.matmul(
                                        ps[:, s * BL:(s + 1) * BL],
                                        whh_sb[:, k2, :,
                                               m * 128:(m + 1) * 128],
                                        h8p_dr[:, k2],
                                        start=(kp == 0 and s == 0
                                               and k2 == 0),
                                        stop=(kp == 1 and s == KH2 - 1
                                              and k2 == KH2 - 1),
                                        perf_mode=DR,
                                        skip_group_check=True,
                                    )
                    ks = slice(hf * KH2, (hf + 1) * KH2)
                    gx_h = gxs[hf]
                    psr = ps_gate[0].rearrange("p (s b) -> p s b", b=BL)
                    psz = ps_gate[1].rearrange("p (s b) -> p s b", b=BL)
                    psn = ps_gate[2]
                    h32h = h32[hf].rearrange("p (k b) -> p k b", b=BL)
                    # r / z gates: descale+add preact onto gx, sigmoid on ACT
                    nc.vector.scalar_tensor_tensor(
                        out=gx_h[:, 0], in0=psr, scalar=ISB, in1=gx_h[:, 0],
                        op0=OP.mult, op1=OP.add)
                    r_h = gate_pool.tile([128, KH2, BL], F16, tag="r_h")
                    nc.scalar.activation(out=r_h, in_=gx_h[:, 0],
                                         func=AF.Sigmoid)
                    nc.vector.scalar_tensor_tensor(
                        out=gx_h[:, 1], in0=psz, scalar=ISB, in1=gx_h[:, 1],
                        op0=OP.mult, op1=OP.add)
                    z_h = gate_pool.tile([128, KH2, BL], F16, tag="z_h")
                    nc.scalar.activation(out=z_h, in_=gx_h[:, 1],
                                         func=AF.Sigmoid)
                    # n gate: tanh(gx_n + r * (gh_n + bhh_n)); psum is x2^14
                    # so bhh_n is pre-scaled x2^14 host-side.
                    for s in range(KH2):
                        kg = hf * KH2 + s
                        nc.vector.scalar_tensor_tensor(
                            out=psn[:, s * BL:(s + 1) * BL],
                            in0=psn[:, s * BL:(s + 1) * BL],
                            scalar=bn_sb[:, kg:kg + 1],
                            in1=r_h[:, s, :],
                            op0=OP.add,
                            op1=OP.mult,
                        )
                    nc.vector.scalar_tensor_tensor(
                        out=gx_h[:, 2],
                        in0=psn.rearrange("p (s b) -> p s b", b=BL),
                        scalar=ISB, in1=gx_h[:, 2],
                        op0=OP.mult, op1=OP.add)
                    n_h = gate_pool.tile([128, KH2, BL], F16, tag="n_h")
                    nc.scalar.activation(out=n_h, in_=gx_h[:, 2],
                                         func=AF.Tanh)
                    # h' = n + z * (h - n)
                    t4 = hops_pool.tile([128, KH2, BL], F16, tag="t4")
                    nc.vector.tensor_sub(t4, h32h, n_h)
                    nc.vector.tensor_mul(t4, z_h, t4)
                    nc.vector.tensor_add(h32h, n_h, t4)
                    nc.vector.tensor_scalar_mul(h8_cur[:, ks], h32h, SH)

                nc.sync.dma_start(
                    out=hall_d[t].rearrange("p (k b) -> p k b", k=KH),
                    in_=h8_cur)
                h8_prev = h8_cur

        # ================= Phase C: logits + log_softmax =================
        with tc.tile_pool(name="c_small", bufs=1) as c_small, \
             tc.tile_pool(name="wout_res", bufs=1) as wo_pool, \
             tc.tile_pool(name="hall_in", bufs=2) as hall_pool, \
             tc.tile_pool(name="logits", bufs=2) as lg_pool, \
             tc.tile_pool(name="expbuf", bufs=2) as ex_pool, \
             tc.tile_pool(name="stats", bufs=8) as st_pool, \
             tc.tile_pool(name="c_ps", bufs=2, space="PSUM") as c_ps:
            bo_sb = c_small.tile([1, E], F16)
            nc.sync.dma_start(out=bo_sb, in_=bo14[:])
            ones_sb = c_small.tile([1, 128], F16)
            nc.vector.memset(ones_sb, 1.0)
            wo_sb = wo_pool.tile([128, KH2, 2, E], F8)
            nc.sync.dma_start(out=wo_sb, in_=wout8_p)
            for mc in range(MC):
                # hs[p, k2, i, tp, b]: lhsT slice [:, k2] -> [128, 2, 128]
                hs = hall_pool.tile([128, KH2, 2, SPM, BL], F8, tag="hs")
                for tp in range(SPM):
                    nc.sync.dma_start(
                        out=hs[:, :, :, tp, :],
                        in_=hall_d[mc * SPM + tp].rearrange(
                            "p (k i b) -> p k i b", i=2, b=BL),
                    )
                lg = lg_pool.tile([128, E], F32, tag="lg")
                for c in range(NCHUNK):
                    ps = c_ps.tile([128, CHUNK], F32, tag="c_psum")
                    for nt in range(NT_CC):
                        nsl = slice(nt * 512, (nt + 1) * 512)
                        nglob = c * CHUNK + nt * 512
                        for k2 in range(KH2):
                            nc.tensor.matmul(
                                ps[:, nsl],
                                hs[:, k2].rearrange("p i t b -> p i (t b)"),
                                wo_sb[:, k2, :, nglob:nglob + 512],
                                start=(k2 == 0),
                                stop=False,
                                perf_mode=DR,
                            )
                        nc.tensor.matmul(
                            ps[:, nsl],
                            ones_sb,
                            bo_sb[:, nglob:nglob + 512],
                            start=False,
                            stop=True,
                        )
                    nc.vector.tensor_scalar_mul(
                        lg[:, c * CHUNK:(c + 1) * CHUNK], ps, ISB)
                negmax = st_pool.tile([128, 1], F32, tag="negmax")
                nc.vector.tensor_reduce(
                    out=negmax, in_=lg, axis=mybir.AxisListType.X,
                    op=OP.max, negate=True)
                eb = ex_pool.tile([128, E], F16, tag="eb")
                sumexp = st_pool.tile([128, 1], F32, tag="sumexp")
                nc.scalar.activation(
                    out=eb, in_=lg, func=AF.Exp,
                    bias=negmax, scale=1.0, accum_out=sumexp)
                lse = st_pool.tile([128, 1], F32, tag="lse")
                nc.scalar.activation(out=lse, in_=sumexp, func=AF.Ln)
                negoff = st_pool.tile([128, 1], F32, tag="negoff")
                nc.vector.tensor_sub(negoff, negmax, lse)
                nc.vector.tensor_scalar_add(lg, lg, negoff)
                nc.sync.dma_start(
                    out=out_d[mc * 128:(mc + 1) * 128, :], in_=lg)

    nc.finalize()
    return nc


def _to_fp8(a, scale):
    return np.clip(a * scale, -240.0, 240.0).astype(NP8)


def _host_prep(context_batch, target_encs, sos, W_ih, W_hh, b_ih, b_hh,
               W_out, b_out, BL, T, H, E):
    """Build per-core input maps (numpy layout transforms only)."""
    G3 = 3 * H
    M3 = G3 // 128
    KH = H // 128
    KH2 = KH // 2
    KE2 = E // 256
    B = context_batch.shape[0]
    ncores = B // BL

    # wih8[m, p, k2, i, c] = W_ih[m*128+c, k2*256+i*128+p] * SW
    wih8 = np.ascontiguousarray(
        _to_fp8(W_ih, SW).reshape(M3, 128, KE2, 2, 128).transpose(
            0, 4, 2, 3, 1)).reshape(M3, 128, -1)
    # whh8[p, k2, i, j] = W_hh[j, (2*k2+i)*128+p] * SW
    whh8 = np.ascontiguousarray(
        _to_fp8(W_hh.T, SW).reshape(KH2, 2, 128, G3).transpose(
            2, 0, 1, 3)).reshape(128, -1)
    # wout8[p, k2, i, e] = W_out[e, (2*k2+i)*128+p] * SW
    wout8 = np.ascontiguousarray(
        _to_fp8(W_out.T, SW).reshape(KH2, 2, 128, E).transpose(
            2, 0, 1, 3)).reshape(128, -1)
    biasA = b_ih.astype(np.float32).copy()
    biasA[:2 * H] += b_hh[:2 * H].astype(np.float32)
    biasA = np.ascontiguousarray(biasA.reshape(M3, 128).T)
    bn14 = np.ascontiguousarray(
        (b_hh[2 * H:].astype(np.float32) * (SW * SH)).reshape(KH, 128).T)
    bo14 = (b_out.astype(np.float32) * (SW * SH)).astype(
        np.float16).reshape(1, E)

    in_maps = []
    for c in range(ncores):
        sl = slice(c * BL, (c + 1) * BL)
        # teacher-forced inputs: SOS, then targets 0..T-2
        xc = np.empty((BL, T, E), np.float32)
        xc[:, 0, :] = sos
        xc[:, 1:, :] = target_encs[sl, :T - 1, :]
        # x8[p, k2, i, n] = X^T[k2*256+i*128+p, t*BL+b] * SX
        xT = xc.transpose(2, 1, 0).reshape(E, T * BL)
        x8 = np.ascontiguousarray(
            _to_fp8(xT, SX).reshape(KE2, 2, 128, T * BL).transpose(
                2, 0, 1, 3)).reshape(128, -1)
        # h0T[p, kk, b] = h0^T[kk*128+p, b]
        h0T = np.ascontiguousarray(
            context_batch[sl].T.reshape(KH, 128, BL).transpose(
                1, 0, 2)).astype(np.float16).reshape(128, -1)
        in_maps.append({
            "x8": x8, "wih8": wih8, "whh8": whh8, "wout8": wout8,
            "h0T": h0T, "biasA": biasA, "bn14": bn14, "bo14": bo14,
        })
    return in_maps


_CACHE = {}


def kernel(context_batch, target_encs, sos, W_ih, W_hh, b_ih, b_hh,
           W_out, b_out, trace=False):
    B, T, E = target_encs.shape
    H = context_batch.shape[1]
    BL = B // NCORES

    if "nc" not in _CACHE:
        _CACHE["nc"] = build(BL=BL, T=T, H=H, E=E)
    nc = _CACHE["nc"]

    in_maps = _host_prep(context_batch, target_encs, sos, W_ih, W_hh,
                         b_ih, b_hh, W_out, b_out, BL, T, H, E)

    from concourse.bass_utils import run_bass_kernel_spmd
    res = run_bass_kernel_spmd(nc, in_maps, list(range(NCORES)), trace=trace)

    outs = []
    for c in range(NCORES):
        o = res.results[c]["out"]            # (T*BL, E), row = t*BL + b
        outs.append(o.reshape(T, BL, E).transpose(1, 0, 2))
    full = np.concatenate(outs, axis=0).astype(np.float32)
    if trace:
        _CACHE["last_exec_time_ns"] = res.exec_time_ns
    return full
